# revision 1
# baseline (speedup 1.0000x reference)
"""Trainium2 Bass kernel for DigitConvolutionalModel:
    out = relu(conv2d_3x3_valid(x.reshape(B,28,28))) .reshape(B,676) @ W + b

Strategy (pure data parallel over 8 cores, B=32768 -> 4096/core):

Per core, samples are processed in 8 supergroups (SG) of 512 = 128
"quads" of 4 consecutive samples. Because consecutive samples are
contiguous in DRAM, a partition-stride-28 DMA over a 512x784 slice lands
a 4-sample "super-image" on 112 partitions:
    X[28k + r, (bq, c)] = x[4*bq + k, 28r + c],  k<4, r<28, bq<128
(one strided DMA per SG; 112-byte contiguous runs). ACT/DVE then cast
fp32 -> fp16.

Conv (cross-correlation) is 3 PSUM-accumulated full-array matmuls per
column-chunk, contracting all 112 rows at once with a block-diagonal
Toeplitz lhsT (M=128, blocks of 26 output rows padded to 32):
    msb[28k + i + di, 128dj + 32k + i] = conv_w[di, dj]
    Y[32k + i, (c, bq)] = sum_dj msb_dj^T @ X[:, (c + dj, bq)]
K=112, M=128, N=512 -> ~91% PE array utilization, no tile_position
tricks. ReLU copies PSUM->SBUF (alternating DVE/ACT) into fp16
h[32k + i, c*128 + bq] (h transposed per sample, c-major).

The FC contracts i for all 4 quad-slots at once with a block-diagonal
W lhsT (K=128, M=128), one matmul per output column c over a PAIR of
supergroups (N=256):
    wsb[32k + i, 128c + 32k + o] = W[26i + c, o]
    outT[32k + o, (s2, bq)] += wsb_c^T @ h[:, (c, s2, bq)]
Bias is added on DVE with a per-partition scalar; small PE transposes
flip outT [10,128] tiles into [128,10] for a contiguous 40B-run store.

All matmul operands are fp16 (e5m10; values are O(10), well in range);
PSUM accumulates fp32, so the only precision loss is ~2^-11 input
rounding.
"""

import sys
import numpy as np

for _p in ("/opt/trn_rl_repo", "/root/.axon_site/_ro/trn_rl_repo"):
    if _p not in sys.path:
        sys.path.insert(0, _p)

import concourse.bass as bass  # noqa: E402,F401
import concourse.tile as tile  # noqa: E402
from concourse import bacc, mybir  # noqa: E402
from concourse.bass_utils import run_bass_kernel_spmd  # noqa: E402

IMG = 28
KW = 3
OUT = 26  # IMG - KW + 1
NPIX = IMG * IMG          # 784
NOUTPIX = OUT * OUT       # 676
NCLS = 10
NCORES = 8
B_TOTAL = 32768
B_CORE = B_TOTAL // NCORES   # 4096
SG = 512                     # samples per supergroup (128 quads of 4)
N_SG = B_CORE // SG          # 8
NQ = 7                       # column chunks: 6x4 + 1x2 = 26 columns
HSTRIDE = OUT * 128          # 3328: per-supergroup h stride in h_pair
F32 = mybir.dt.float32
F16 = mybir.dt.float16

_CACHE = {}


def _chunk_cols(q):
    """(first output column, n columns) of chunk q."""
    return 4 * q, (2 if q == NQ - 1 else 4)


def _build_program(mm_dtype=F16, n_sg=N_SG, rep=1, hwloop=0, stage=5):
    """Build + compile the per-core Bass program (identical on all cores)."""
    nc = bacc.Bacc("TRN2", target_bir_lowering=False, debug=False,
                   num_devices=NCORES)

    x_d = nc.dram_tensor("x", (N_SG, 112, 128 * IMG), mm_dtype,
                         kind="ExternalInput")
    msb_d = nc.dram_tensor("msb", (128, 3 * 128), mm_dtype,
                           kind="ExternalInput")
    wsb_d = nc.dram_tensor("wsb", (128, OUT * 128), mm_dtype,
                           kind="ExternalInput")
    bias_d = nc.dram_tensor("biasv", (128, 1), F32, kind="ExternalInput")
    id_d = nc.dram_tensor("ident", (128, 32), F32, kind="ExternalInput")
    out_d = nc.dram_tensor("out", (B_CORE, NCLS), F32, kind="ExternalOutput")

    x_ap = x_d.ap()
    out_ap = out_d.ap()

    with tile.TileContext(nc) as tc:
        with (
            tc.tile_pool(name="consts", bufs=1) as consts,
            tc.tile_pool(name="xin", bufs=8) as xin,
            tc.tile_pool(name="hbuf", bufs=3) as hbuf,
            tc.tile_pool(name="obuf", bufs=2) as obuf,
            tc.tile_pool(name="convps", bufs=6, space="PSUM") as convps,
            tc.tile_pool(name="fcps", bufs=2, space="PSUM") as fcps,
        ):
            msb = consts.tile([128, 3 * 128], mm_dtype)
            wsb = consts.tile([128, OUT * 128], mm_dtype)
            biasv = consts.tile([128, 1], F32)
            ident = consts.tile([128, 32], F32)
            nc.sync.dma_start(out=msb[:, :], in_=msb_d.ap())
            nc.sync.dma_start(out=wsb[:, :], in_=wsb_d.ap())
            nc.sync.dma_start(out=biasv[:, :], in_=bias_d.ap())
            nc.sync.dma_start(out=ident[:, :], in_=id_d.ap())

            import contextlib
            loop_cm = (tc.For_i(0, hwloop, 1) if hwloop
                       else contextlib.nullcontext())
            with loop_cm:
              h_pair = None
              for s in [s_ for _ in range(rep) for s_ in range(n_sg)]:
                # ---- load supergroup s (split across both HWDGE rings) ----
                # host pre-permuted: x_d[s, 28k + r, c*128 + bq]
                #   = fp16(x[...]); c-major so conv rhs columns are
                # 256B-contiguous in b
                xt = xin.tile([128, 128 * IMG], mm_dtype, tag="xt")
                eng = nc.sync if s % 2 == 0 else nc.scalar
                eng.dma_start(out=xt[0:112, :], in_=x_ap[s])

                if stage < 2:
                    dmy = obuf.tile([128, 8], F32, tag="dmy")
                    nc.vector.tensor_copy(dmy[0:1, 0:8],
                                          xt[0:1, 0:16].bitcast(F32))
                h_sg = hbuf.tile([128, HSTRIDE], mm_dtype, tag="h")
                # free layout: c*128 + bq  (3328 = 26*128)

                # ---- conv: per column-chunk q, 3 accumulated matmuls ----
                xv = xt[0:112, :].rearrange("p (c b) -> p c b", b=128)
                if stage >= 3:
                    pass
                for q in range(NQ if (stage >= 2 or stage == 6) else 0):
                    c0, ncol = _chunk_cols(q)
                    pq = convps.tile([128, 512], F32, tag="pq")
                    for dj in range(3):
                        nc.tensor.matmul(
                            pq[0:128, 0:ncol * 128],
                            msb[0:112, 128 * dj:128 * dj + 128],
                            xv[:, c0 + dj:c0 + dj + ncol, :],
                            start=(dj == 0), stop=(dj == 2),
                        )
                    # ---- relu PSUM -> SBUF (h transposed, c-major) ----
                    if stage < 3 or stage == 6:
                        continue
                    hslice = h_sg[:, c0 * 128:(c0 + ncol) * 128]
                    if (s * NQ + q) % 2 == 0:
                        nc.vector.tensor_scalar_max(
                            hslice, pq[:, 0:ncol * 128], 0.0)
                    else:
                        nc.scalar.activation(
                            hslice, pq[:, 0:ncol * 128],
                            mybir.ActivationFunctionType.Relu)

                # ---- FC + bias + transpose + store, one SG behind ----
                def fc_block(s, h_sg):
                    ot = fcps.tile([128, 128], F32, tag="ot")
                    hv = h_sg[:, :].rearrange("p (c b) -> p c b", b=128)
                    for c in range(OUT):
                        nc.tensor.matmul(
                            ot[0:128, 0:128],
                            wsb[0:128, 128 * c:128 * c + 128],
                            hv[:, c, :],
                            start=(c == 0), stop=(c == OUT - 1),
                        )
                    # bias add (per-partition scalar) PSUM -> SBUF
                    osb = obuf.tile([128, 128], F32, tag="osb")
                    nc.vector.tensor_scalar_add(osb[:, :], ot[:, :],
                                                biasv[:, 0:1])
                    if stage < 5:
                        return
                    # DVE 32x32 block transpose: osb[32k+o, bq] ->
                    # tt[32k + bq%32, (bq//32)*32 + o]
                    tt_t = obuf.tile([128, 128], F32, tag="ttbuf")
                    for k in range(4):
                        nc.vector.transpose(tt_t[32 * k:32 * k + 32, 0:128],
                                            osb[32 * k:32 * k + 32, 0:128])
                    # gather the valid o<10 columns of each 32-block into
                    # outsb[:, s*40 + hi*10 + o]
                    nc.vector.tensor_copy(
                        outsb[:, s * 40:(s + 1) * 40].rearrange(
                            "p (hi o) -> p hi o", hi=4),
                        tt_t[:, :].rearrange(
                            "p (hi o) -> p hi o", hi=4)[:, :, 0:NCLS])
                    if s == n_sg - 1:
                        # sample n = (32k + bq%32)*32 + 4s + bq//32:
                        # one DMA, 1280B contiguous runs per partition
                        nc.scalar.dma_start(
                            out=out_ap[:, :].rearrange(
                                "(p ss hi) o -> p ss hi o", ss=8, hi=4),
                            in_=outsb[:, :].rearrange(
                                "p (ss hi o) -> p ss hi o", ss=8, hi=4))

                if stage >= 4 and stage != 6:
                    if s == 0:
                        outsb = obuf.tile([128, 32 * NCLS], F32, tag="outsb")
                        prev = None
                    if prev is not None:
                        fc_block(*prev)
                    prev = (s, h_sg)
                    if s == n_sg - 1:
                        fc_block(*prev)

    nc.compile()
    return nc


def _host_constants(conv_w, W, b):
    """Block-diagonal Toeplitz conv lhsT and block-diagonal FC lhsT."""
    msb = np.zeros((128, 3 * 128), np.float32)
    for dj in range(KW):
        for k in range(4):
            for i in range(OUT):
                for di in range(KW):
                    msb[28 * k + i + di, 128 * dj + 32 * k + i] = conv_w[di, dj]
    wsb = np.zeros((128, OUT * 128), np.float32)
    for c in range(OUT):
        for k in range(4):
            for i in range(OUT):
                wsb[32 * k + i, 128 * c + 32 * k:128 * c + 32 * k + NCLS] = \
                    W[i * OUT + c, :]
    biasv = np.zeros((128, 1), np.float32)
    for k in range(4):
        biasv[32 * k:32 * k + NCLS, 0] = b
    ident = np.zeros((128, 32), np.float32)
    for p in range(128):
        ident[p, p % 32] = 1.0
    return msb, wsb, biasv, ident


def _marshal_x(x):
    """[B, 784] fp32 -> per-core [N_SG, 112, 3584] fp16 stacked layout."""
    # sample n = (32k + bq%32)*32 + 4s + bq//32; bq = 32*hi + bl
    # n axes: [k(4), bl(32), s(8), hi(4)]
    xs = x.reshape(NCORES, 4, 32, 8, 4, IMG, IMG)     # core k bl s hi r c
    xs = xs.transpose(0, 3, 1, 5, 6, 4, 2)            # core s k r c hi bl
    xs = np.ascontiguousarray(xs, dtype=np.float16)
    return xs.reshape(NCORES, N_SG, 112, 128 * IMG)


def _run(x, conv_w, W, b, trace=False, mm_dtype=F16):
    x = np.ascontiguousarray(np.asarray(x, dtype=np.float32))
    conv_w = np.asarray(conv_w, dtype=np.float32)
    W = np.asarray(W, dtype=np.float32)
    b = np.asarray(b, dtype=np.float32)
    assert x.shape == (B_TOTAL, NPIX), x.shape

    key = ("prog", str(mm_dtype))
    if key not in _CACHE:
        _CACHE[key] = _build_program(mm_dtype)
    nc = _CACHE[key]

    msb, wsb, biasv, ident = _host_constants(conv_w, W, b)
    msb_r, wsb_r = msb.astype(np.float16), wsb.astype(np.float16)
    xm = _marshal_x(x)
    in_maps = []
    for i in range(NCORES):
        in_maps.append({
            "x": xm[i],
            "msb": msb_r, "wsb": wsb_r, "biasv": biasv, "ident": ident,
        })
    res = run_bass_kernel_spmd(nc, in_maps, core_ids=list(range(NCORES)),
                               trace=trace)
    out = np.concatenate([res.results[i]["out"] for i in range(NCORES)],
                         axis=0)
    return out, res


def kernel(x, conv_w, W, b):
    out, _ = _run(x, conv_w, W, b, trace=False)
    return out

